# revision 1
# baseline (speedup 1.0000x reference)
"""Trainium2 Bass kernel for nn_CompMLP (embedding gathers + 3-layer MLP).

Strategy (pure data parallel, 8 cores, B rows split evenly):
  - All embedding gathers run on-device via GPSIMD ap_gather from
    SBUF-resident tables, in bf16 with d=2 (one 32-bit word per index per
    partition; partition p holds dim-pair (2q, 2q+1)).
  - A host-precomputed pair-sum table  S2[i*171+j] = emb[i]+emb[j]  lets the
    9 ally/enem lookups collapse to 4 pair lookups; the remaining per-row
    sums happen for free in PSUM accumulation (matmul cost is independent
    of K).
  - Gathered tiles feed the MLP directly in transposed (feature-on-
    partition) layout: even/odd stride-2 matmuls, fp32 PSUM accumulate,
    ScalarE fuses bias+ReLU on PSUM->SBUF eviction.

Layout per 512-row tile:
  T1 [128p x 512] <- ap_gather(A): 4 lists (a01, a23, e01, e23), 32
     partitions each, from the pair-sum champ table (29241 elems).
  T2 [128p x 512] <- ap_gather(B): lists (my, my, e4, e4, m01, m23, pat,
     junk) per 16-partition group from singles/misc-concat tables.
  h1[256] = relu(sum of 8 matmuls + b1); h2 = relu(2 matmuls + b2);
  out = 1 matmul + b3.
"""

import numpy as np
import ml_dtypes

import concourse.bass as bass  # noqa: F401  (engine types referenced via nc)
import concourse.mybir as mybir
from concourse import bacc
from concourse.tile import TileContext
from concourse.bass_utils import run_bass_kernel_spmd

# ---- problem constants (hardcoded per contract) ----
B_TOTAL = 262144
NCHAMP = 171
DC = 64
DM = 16
MISC_V = (33, 9, 9, 65, 65)
N_CORES = 8
B_CORE = B_TOTAL // N_CORES  # 32768

F = 512                      # batch rows per tile
T_TILES = B_CORE // F        # 64

NE_A = NCHAMP * NCHAMP       # 29241 pair-sum elems
NE_B = 585                   # max elems in the singles/misc buffer

BF16 = mybir.dt.bfloat16
F32 = mybir.dt.float32
I16 = mybir.dt.int16
AF = mybir.ActivationFunctionType

_COMPILED = {}


def _fix(x, n):
    return np.where(x < 0, n - 1, x).astype(np.int64)


def _pair_layout(tab):
    """[rows, 2*P] table -> [P, rows, 2] partition-pair layout (bf16)."""
    rows, dims = tab.shape
    assert dims % 2 == 0
    t = tab.astype(ml_dtypes.bfloat16).reshape(rows, dims // 2, 2)
    return np.ascontiguousarray(t.transpose(1, 0, 2))


def _wrap_idx(lists):
    """8 per-group idx lists [B_CORE] -> [128, T_TILES*(F//16)] int16 wrapped,
    tiles side by side along the free dim."""
    out = np.zeros((T_TILES, 128, F // 16), dtype=np.int16)
    for g, lst in enumerate(lists):
        w = lst.reshape(T_TILES, F // 16, 16).transpose(0, 2, 1)
        out[:, g * 16:(g + 1) * 16, :] = w
    return np.ascontiguousarray(
        out.transpose(1, 0, 2).reshape(128, T_TILES * (F // 16)))


def _build_program():
    nc = bacc.Bacc("TRN2", target_bir_lowering=False, debug=False,
                   num_devices=N_CORES)

    A_d = nc.dram_tensor("tabA", [128, NE_A * 2], BF16, kind="ExternalInput")
    B_d = nc.dram_tensor("tabB", [128, NE_B * 2], BF16, kind="ExternalInput")
    i1_d = nc.dram_tensor("idx1", [128, T_TILES * (F // 16)], I16,
                          kind="ExternalInput")
    i2_d = nc.dram_tensor("idx2", [128, T_TILES * (F // 16)], I16,
                          kind="ExternalInput")
    w1_d = nc.dram_tensor("w1", [4, 2, 128, 128], BF16, kind="ExternalInput")
    w2_d = nc.dram_tensor("w2", [2, 128, 128], BF16, kind="ExternalInput")
    w3_d = nc.dram_tensor("w3", [128, 1], BF16, kind="ExternalInput")
    b1_d = nc.dram_tensor("b1", [2, 128, 1], F32, kind="ExternalInput")
    b2_d = nc.dram_tensor("b2", [128, 1], F32, kind="ExternalInput")
    b3_d = nc.dram_tensor("b3", [1, 1], F32, kind="ExternalInput")
    out_d = nc.dram_tensor("out", [T_TILES, F], F32, kind="ExternalOutput")

    with TileContext(nc) as tc:
        with (
            tc.tile_pool(name="const", bufs=1) as cpool,
            tc.tile_pool(name="gath", bufs=4) as gpool,
            tc.tile_pool(name="act", bufs=3) as hpool,
            tc.tile_pool(name="outp", bufs=8) as opool,
            tc.tile_pool(name="ps1", bufs=3, space="PSUM") as ps1pool,
            tc.tile_pool(name="ps2", bufs=2, space="PSUM") as ps2pool,
        ):
            A_t = cpool.tile([128, NE_A * 2], BF16, tag="tabA")
            nc.sync.dma_start(out=A_t[:, :], in_=A_d[:, :])
            B_t = cpool.tile([128, NE_B * 2], BF16, tag="tabB")
            nc.sync.dma_start(out=B_t[:, :], in_=B_d[:, :])
            w1_t = [[cpool.tile([128, 128], BF16, tag=f"w1_{s}_{m}", name=f"w1_{s}_{m}")
                     for m in range(2)] for s in range(4)]
            for s in range(4):
                for m in range(2):
                    nc.sync.dma_start(out=w1_t[s][m][:, :], in_=w1_d[s, m])
            w2_t = [cpool.tile([128, 128], BF16, tag=f"w2_{m}", name=f"w2_{m}")
                    for m in range(2)]
            for m in range(2):
                nc.sync.dma_start(out=w2_t[m][:, :], in_=w2_d[m])
            w3_t = cpool.tile([128, 1], BF16, tag="w3")
            nc.sync.dma_start(out=w3_t[:, :], in_=w3_d[:, :])
            b1_t = [cpool.tile([128, 1], F32, tag=f"b1_{m}", name=f"b1_{m}") for m in range(2)]
            for m in range(2):
                nc.sync.dma_start(out=b1_t[m][:, :], in_=b1_d[m])
            b2_t = cpool.tile([128, 1], F32, tag="b2")
            nc.sync.dma_start(out=b2_t[:, :], in_=b2_d[:, :])
            b3_t = cpool.tile([1, 1], F32, tag="b3")
            nc.sync.dma_start(out=b3_t[:, :], in_=b3_d[:, :])
            i1_all = cpool.tile([128, T_TILES * (F // 16)], I16, tag="i1a")
            nc.sync.dma_start(out=i1_all[:, :], in_=i1_d[:, :])
            i2_all = cpool.tile([128, T_TILES * (F // 16)], I16, tag="i2a")
            nc.sync.dma_start(out=i2_all[:, :], in_=i2_d[:, :])

            G = F // 16
            for t in range(T_TILES):
                g1 = gpool.tile([128, 2 * F], BF16, tag="g1")
                nc.gpsimd.ap_gather(g1[:, :], A_t[:, :],
                                    i1_all[:, t * G:(t + 1) * G],
                                    channels=128, num_elems=NE_A, d=2,
                                    num_idxs=F)
                g2 = gpool.tile([128, 2 * F], BF16, tag="g2")
                nc.gpsimd.ap_gather(g2[:, :], B_t[:, :],
                                    i2_all[:, t * G:(t + 1) * G],
                                    channels=128, num_elems=NE_B, d=2,
                                    num_idxs=F)
                g1r = g1[:, :].rearrange("p (f d) -> p f d", d=2)
                g2r = g2[:, :].rearrange("p (f d) -> p f d", d=2)

                h1 = []
                for m in range(2):
                    ps = ps1pool.tile([128, F], F32, tag="ps1")
                    nc.tensor.matmul(ps[:, :], w1_t[0][m][:, :], g1r[:, :, 0],
                                     start=True, stop=False)
                    nc.tensor.matmul(ps[:, :], w1_t[1][m][:, :], g1r[:, :, 1],
                                     start=False, stop=False)
                    nc.tensor.matmul(ps[:, :], w1_t[2][m][:, :], g2r[:, :, 0],
                                     start=False, stop=False)
                    nc.tensor.matmul(ps[:, :], w1_t[3][m][:, :], g2r[:, :, 1],
                                     start=False, stop=True)
                    hm = hpool.tile([128, F], BF16, tag=f"h1_{m}")
                    nc.scalar.activation(hm[:, :], ps[:, :], AF.Relu,
                                         bias=b1_t[m][:, 0:1])
                    h1.append(hm)

                ps2 = ps1pool.tile([128, F], F32, tag="ps2")
                nc.tensor.matmul(ps2[:, :], w2_t[0][:, :], h1[0][:, :],
                                 start=True, stop=False)
                nc.tensor.matmul(ps2[:, :], w2_t[1][:, :], h1[1][:, :],
                                 start=False, stop=True)
                h2 = hpool.tile([128, F], BF16, tag="h2")
                nc.scalar.activation(h2[:, :], ps2[:, :], AF.Relu,
                                     bias=b2_t[:, 0:1])

                ps3 = ps2pool.tile([1, F], F32, tag="ps3")
                nc.tensor.matmul(ps3[:, :], w3_t[:, 0:1], h2[:, :],
                                 start=True, stop=True)
                ot = opool.tile([1, F], F32, tag="ot")
                nc.scalar.activation(ot[:, :], ps3[:, :], AF.Identity,
                                     bias=b3_t[0:1, 0:1])
                nc.sync.dma_start(out=out_d[t:t + 1, :], in_=ot[:, :])

    nc.compile()
    return nc


def _prep_inputs(my_idx, ally, enem, misc_idx, emb_champ, emb_sp, emb_pri,
                 emb_sub, emb_key, emb_pat, W1, b1, W2, b2, W3, b3):
    emb = np.asarray(emb_champ, np.float32)

    # --- tables ---
    pair = (emb[:, None, :] + emb[None, :, :]).reshape(NE_A, DC)
    blkA = _pair_layout(pair)                      # [32, NE_A, 2]
    A_arr = np.ascontiguousarray(
        np.broadcast_to(blkA[None], (4, 32, NE_A, 2)).reshape(128, NE_A * 2))

    B_arr = np.zeros((128, NE_B, 2), dtype=ml_dtypes.bfloat16)
    sing = _pair_layout(emb)                       # [32, 171, 2]
    B_arr[0:32, :NCHAMP] = sing
    B_arr[32:64, :NCHAMP] = sing
    m01 = np.concatenate(
        [np.repeat(np.asarray(emb_sp, np.float32), MISC_V[1], 0),
         np.tile(np.asarray(emb_pri, np.float32), (MISC_V[0], 1))], axis=1)
    B_arr[64:80, :m01.shape[0]] = _pair_layout(m01)
    m23 = np.concatenate(
        [np.repeat(np.asarray(emb_sub, np.float32), MISC_V[3], 0),
         np.tile(np.asarray(emb_key, np.float32), (MISC_V[2], 1))], axis=1)
    B_arr[80:96, :m23.shape[0]] = _pair_layout(m23)
    pat = np.concatenate([np.asarray(emb_pat, np.float32),
                          np.zeros((MISC_V[4], DM), np.float32)], axis=1)
    B_arr[96:112, :MISC_V[4]] = _pair_layout(pat)
    B_arr = np.ascontiguousarray(B_arr.reshape(128, NE_B * 2))

    # --- weights ---
    W1z = np.concatenate([np.asarray(W1, np.float32),
                          np.zeros((1, 256), np.float32)], axis=0)
    q = np.arange(32)
    t1e = np.concatenate([64 + 2 * q, 64 + 2 * q, 128 + 2 * q, 128 + 2 * q])
    t1o = t1e + 1
    qa = np.arange(16)
    pat_e = np.where(2 * qa < DM, 256 + 2 * qa, 272)
    pat_o = np.where(2 * qa + 1 < DM, 257 + 2 * qa, 272)
    t2e = np.concatenate([2 * q, 128 + 2 * q, 192 + 2 * qa, 224 + 2 * qa,
                          pat_e, np.full(16, 272)])
    t2o = np.concatenate([2 * q + 1, 129 + 2 * q, 193 + 2 * qa, 225 + 2 * qa,
                          pat_o, np.full(16, 272)])
    w1_arr = np.zeros((4, 2, 128, 128), dtype=ml_dtypes.bfloat16)
    for s, rows in enumerate([t1e, t1o, t2e, t2o]):
        sel = W1z[rows]                             # [128, 256]
        for m in range(2):
            w1_arr[s, m] = sel[:, m * 128:(m + 1) * 128]
    w2_arr = np.asarray(W2, np.float32).astype(ml_dtypes.bfloat16)
    w2_arr = np.ascontiguousarray(w2_arr.reshape(2, 128, 128))
    w3_arr = np.asarray(W3, np.float32).astype(ml_dtypes.bfloat16)
    b1_arr = np.asarray(b1, np.float32).reshape(2, 128, 1)
    b2_arr = np.asarray(b2, np.float32).reshape(128, 1)
    b3_arr = np.asarray(b3, np.float32).reshape(1, 1)

    # --- indices ---
    myx = _fix(np.asarray(my_idx), NCHAMP)
    al = _fix(np.asarray(ally), NCHAMP)
    en = _fix(np.asarray(enem), NCHAMP)
    mi = np.asarray(misc_idx)
    mif = [_fix(mi[:, j], MISC_V[j]) for j in range(5)]

    a01 = al[:, 0] * NCHAMP + al[:, 1]
    a23 = al[:, 2] * NCHAMP + al[:, 3]
    e01 = en[:, 0] * NCHAMP + en[:, 1]
    e23 = en[:, 2] * NCHAMP + en[:, 3]
    m01i = mif[0] * MISC_V[1] + mif[1]
    m23i = mif[2] * MISC_V[3] + mif[3]
    zero = np.zeros(B_TOTAL, np.int64)

    l1 = [a01, a01, a23, a23, e01, e01, e23, e23]
    l2 = [myx, myx, en[:, 4], en[:, 4], m01i, m23i, mif[4], zero]

    in_maps = []
    for c in range(N_CORES):
        s = slice(c * B_CORE, (c + 1) * B_CORE)
        in_maps.append({
            "tabA": A_arr, "tabB": B_arr,
            "idx1": _wrap_idx([x[s].astype(np.int16) for x in l1]),
            "idx2": _wrap_idx([x[s].astype(np.int16) for x in l2]),
            "w1": w1_arr, "w2": w2_arr, "w3": w3_arr,
            "b1": b1_arr, "b2": b2_arr, "b3": b3_arr,
        })
    return in_maps


def kernel(**inputs):
    if "nc" not in _COMPILED:
        _COMPILED["nc"] = _build_program()
    nc = _COMPILED["nc"]
    in_maps = _prep_inputs(**inputs)
    res = run_bass_kernel_spmd(nc, in_maps, core_ids=list(range(N_CORES)))
    out = np.concatenate([r["out"].reshape(B_CORE) for r in res.results])
    return out.astype(np.float32)



# revision 2
# speedup vs baseline: 2.2818x; 2.2818x over previous
"""Trainium2 Bass kernel for nn_CompMLP (embedding gathers + 3-layer MLP).

Strategy v2 (pure data parallel, 8 cores, B rows split evenly):
  - All embedding gathers are DMA gathers (SWDGE) from HBM tables of
    256-byte rows, in transposed mode so each gathered row lands
    feature-on-partition, ready for matmul.  GPSIMD only generates DMA
    descriptors (994ns + 0.34ns/desc per call) instead of doing the
    gather itself (the old ap_gather path cost ~15us per 512 indices).
  - Tables (per-core HBM, built on host):
      t_pair [14706, 128]: [emb_i | emb_j] for sorted champ pairs
        (i>=j, k=i(i+1)/2+j).  Serves a01/a23/e01/e23; the ally-vs-enemy
        distinction lives in the stationary weights (W1 slice stacked
        twice), and pair-sum happens for free in PSUM accumulation.
      t_mye4 [29241, 128]: [emb_my | emb_e4], k = my*171 + e4.
      t_misc [7410, 128]: rows 0..5264   [e_pri|e_sub|e_pat|0pad],
                          rows 5265..7409 [e_sp|e_key|0pad].
  - Per 2048-row supertile: 3 dma_gather calls (pairs: 8192 idxs,
    mye4: 2048, misc: 4096), then 4x 512-row MLP subtiles: 7 K-chunk
    matmuls -> 256-dim h1 (ScalarE fused bias+ReLU), 2 matmuls -> h2,
    1 matmul -> out.
"""

import numpy as np
import ml_dtypes

import concourse.bass as bass  # noqa: F401
import concourse.mybir as mybir
from concourse import bacc
from concourse.tile import TileContext
from concourse.bass_utils import run_bass_kernel_spmd

# ---- problem constants (hardcoded per contract) ----
B_TOTAL = 262144
NCHAMP = 171
DC = 64
DM = 16
MISC_V = (33, 9, 9, 65, 65)
N_CORES = 8
B_CORE = B_TOTAL // N_CORES   # 32768

G = 2048                      # supertile rows
NSUP = B_CORE // G            # 16
F = 512                       # MLP subtile rows
NSUB = G // F                 # 4

NPAIR = NCHAMP * (NCHAMP + 1) // 2   # 14706 sorted pairs
NMYE4 = NCHAMP * NCHAMP              # 29241
NMA = MISC_V[1] * MISC_V[2] * MISC_V[4]   # 5265 (pri, sub, pat)
NMB = MISC_V[0] * MISC_V[3]               # 2145 (sp, key)
NMISC = NMA + NMB                         # 7410

BF16 = mybir.dt.bfloat16
F32 = mybir.dt.float32
I16 = mybir.dt.int16
AF = mybir.ActivationFunctionType

_COMPILED = {}


def _fix(x, n):
    return np.where(x < 0, n - 1, x).astype(np.int64)


def _wrap16(idx):
    """[N] index list -> [128, N//16] int16, wrapped in 16 partitions and
    replicated across the 8 GPSIMD cores (dma_gather index layout)."""
    n = idx.shape[0]
    w = idx.reshape(n // 16, 16).T.astype(np.int16)   # [16, N/16]
    return np.tile(w, (8, 1))                          # [128, N/16]


def _build_program():
    nc = bacc.Bacc("TRN2", target_bir_lowering=False, debug=False,
                   num_devices=N_CORES)

    tp_d = nc.dram_tensor("t_pair", [NPAIR, 128], BF16, kind="ExternalInput")
    tm_d = nc.dram_tensor("t_mye4", [NMYE4, 128], BF16, kind="ExternalInput")
    tc_d = nc.dram_tensor("t_misc", [NMISC, 128], BF16, kind="ExternalInput")
    iA_d = nc.dram_tensor("idxA", [128, NSUP * (4 * G // 16)], I16,
                          kind="ExternalInput")
    iB_d = nc.dram_tensor("idxB", [128, NSUP * (G // 16)], I16,
                          kind="ExternalInput")
    iC_d = nc.dram_tensor("idxC", [128, NSUP * (2 * G // 16)], I16,
                          kind="ExternalInput")
    w1_d = nc.dram_tensor("w1", [5, 2, 128, 128], BF16, kind="ExternalInput")
    w2_d = nc.dram_tensor("w2", [2, 128, 128], BF16, kind="ExternalInput")
    w3_d = nc.dram_tensor("w3", [128, 1], BF16, kind="ExternalInput")
    b1_d = nc.dram_tensor("b1", [2, 128, 1], F32, kind="ExternalInput")
    b2_d = nc.dram_tensor("b2", [128, 1], F32, kind="ExternalInput")
    b3_d = nc.dram_tensor("b3", [1, 1], F32, kind="ExternalInput")
    out_d = nc.dram_tensor("out", [B_CORE // F, F], F32, kind="ExternalOutput")

    cA = 4 * G // 16   # idx cols per supertile, call A (512)
    cB = G // 16       # 128
    cC = 2 * G // 16   # 256

    with TileContext(nc) as tc:
        with (
            tc.tile_pool(name="const", bufs=1) as cpool,
            tc.tile_pool(name="gath", bufs=2) as gpool,
            tc.tile_pool(name="act", bufs=3) as hpool,
            tc.tile_pool(name="outp", bufs=8) as opool,
            tc.tile_pool(name="ps1", bufs=4, space="PSUM") as ps1pool,
            tc.tile_pool(name="ps2", bufs=2, space="PSUM") as ps2pool,
            tc.tile_pool(name="ps3", bufs=2, space="PSUM") as ps3pool,
        ):
            iA_t = cpool.tile([128, NSUP * cA], I16, tag="iA")
            nc.sync.dma_start(out=iA_t[:, :], in_=iA_d[:, :])
            iB_t = cpool.tile([128, NSUP * cB], I16, tag="iB")
            nc.sync.dma_start(out=iB_t[:, :], in_=iB_d[:, :])
            iC_t = cpool.tile([128, NSUP * cC], I16, tag="iC")
            nc.sync.dma_start(out=iC_t[:, :], in_=iC_d[:, :])
            w1_t = [[cpool.tile([128, 128], BF16, tag=f"w1_{k}_{m}",
                                name=f"w1_{k}_{m}")
                     for m in range(2)] for k in range(5)]
            for k in range(5):
                for m in range(2):
                    nc.sync.dma_start(out=w1_t[k][m][:, :], in_=w1_d[k, m])
            w2_t = [cpool.tile([128, 128], BF16, tag=f"w2_{m}", name=f"w2_{m}")
                    for m in range(2)]
            for m in range(2):
                nc.sync.dma_start(out=w2_t[m][:, :], in_=w2_d[m])
            w3_t = cpool.tile([128, 1], BF16, tag="w3")
            nc.sync.dma_start(out=w3_t[:, :], in_=w3_d[:, :])
            b1_t = [cpool.tile([128, 1], F32, tag=f"b1_{m}", name=f"b1_{m}")
                    for m in range(2)]
            for m in range(2):
                nc.sync.dma_start(out=b1_t[m][:, :], in_=b1_d[m])
            b2_t = cpool.tile([128, 1], F32, tag="b2")
            nc.sync.dma_start(out=b2_t[:, :], in_=b2_d[:, :])
            b3_t = cpool.tile([1, 1], F32, tag="b3")
            nc.sync.dma_start(out=b3_t[:, :], in_=b3_d[:, :])

            for s in range(NSUP):
                pg = gpool.tile([128, 1, 4 * G], BF16, tag="pg")
                nc.gpsimd.dma_gather(
                    pg[:, :, :], tp_d[:, :],
                    iA_t[:, s * cA:(s + 1) * cA],
                    num_idxs=4 * G, num_idxs_reg=4 * G,
                    elem_size=128, transpose=True)
                mg = gpool.tile([128, 1, G], BF16, tag="mg")
                nc.gpsimd.dma_gather(
                    mg[:, :, :], tm_d[:, :],
                    iB_t[:, s * cB:(s + 1) * cB],
                    num_idxs=G, num_idxs_reg=G,
                    elem_size=128, transpose=True)
                cg = gpool.tile([128, 1, 2 * G], BF16, tag="cg")
                nc.gpsimd.dma_gather(
                    cg[:, :, :], tc_d[:, :],
                    iC_t[:, s * cC:(s + 1) * cC],
                    num_idxs=2 * G, num_idxs_reg=2 * G,
                    elem_size=128, transpose=True)

                for t in range(NSUB):
                    c0 = t * F
                    h1 = []
                    for m in range(2):
                        ps = ps1pool.tile([128, F], F32, tag="ps1")
                        nc.tensor.matmul(ps[:, :], w1_t[0][m][:, :],
                                         pg[:, 0, c0:c0 + F],
                                         start=True, stop=False)
                        nc.tensor.matmul(ps[:, :], w1_t[0][m][:, :],
                                         pg[:, 0, G + c0:G + c0 + F],
                                         start=False, stop=False)
                        nc.tensor.matmul(ps[:, :], w1_t[1][m][:, :],
                                         pg[:, 0, 2 * G + c0:2 * G + c0 + F],
                                         start=False, stop=False)
                        nc.tensor.matmul(ps[:, :], w1_t[1][m][:, :],
                                         pg[:, 0, 3 * G + c0:3 * G + c0 + F],
                                         start=False, stop=False)
                        nc.tensor.matmul(ps[:, :], w1_t[2][m][:, :],
                                         mg[:, 0, c0:c0 + F],
                                         start=False, stop=False)
                        nc.tensor.matmul(ps[:, :], w1_t[3][m][:, :],
                                         cg[:, 0, c0:c0 + F],
                                         start=False, stop=False)
                        nc.tensor.matmul(ps[:, :], w1_t[4][m][:, :],
                                         cg[:, 0, G + c0:G + c0 + F],
                                         start=False, stop=True)
                        hm = hpool.tile([128, F], BF16, tag=f"h1_{m}")
                        nc.scalar.activation(hm[:, :], ps[:, :], AF.Relu,
                                             bias=b1_t[m][:, 0:1])
                        h1.append(hm)

                    ps2 = ps2pool.tile([128, F], F32, tag="ps2")
                    nc.tensor.matmul(ps2[:, :], w2_t[0][:, :], h1[0][:, :],
                                     start=True, stop=False)
                    nc.tensor.matmul(ps2[:, :], w2_t[1][:, :], h1[1][:, :],
                                     start=False, stop=True)
                    h2 = hpool.tile([128, F], BF16, tag="h2")
                    nc.scalar.activation(h2[:, :], ps2[:, :], AF.Relu,
                                         bias=b2_t[:, 0:1])

                    ps3 = ps3pool.tile([1, F], F32, tag="ps3")
                    nc.tensor.matmul(ps3[:, :], w3_t[:, 0:1], h2[:, :],
                                     start=True, stop=True)
                    ot = opool.tile([1, F], F32, tag="ot")
                    nc.scalar.activation(ot[:, :], ps3[:, :], AF.Identity,
                                         bias=b3_t[0:1, 0:1])
                    row = s * NSUB + t
                    nc.sync.dma_start(out=out_d[row:row + 1, :], in_=ot[:, :])

    nc.compile()
    return nc


def _prep_inputs(my_idx, ally, enem, misc_idx, emb_champ, emb_sp, emb_pri,
                 emb_sub, emb_key, emb_pat, W1, b1, W2, b2, W3, b3):
    emb = np.asarray(emb_champ, np.float32)
    e_sp = np.asarray(emb_sp, np.float32)
    e_pri = np.asarray(emb_pri, np.float32)
    e_sub = np.asarray(emb_sub, np.float32)
    e_key = np.asarray(emb_key, np.float32)
    e_pat = np.asarray(emb_pat, np.float32)

    # --- tables ---
    ii, jj = np.tril_indices(NCHAMP)          # k = i*(i+1)/2 + j, i >= j
    t_pair = np.concatenate([emb[ii], emb[jj]], axis=1)          # [14706,128]
    my_g = np.repeat(np.arange(NCHAMP), NCHAMP)
    e4_g = np.tile(np.arange(NCHAMP), NCHAMP)
    t_mye4 = np.concatenate([emb[my_g], emb[e4_g]], axis=1)      # [29241,128]
    pri_g = np.repeat(np.arange(MISC_V[1]), MISC_V[2] * MISC_V[4])
    sub_g = np.tile(np.repeat(np.arange(MISC_V[2]), MISC_V[4]), MISC_V[1])
    pat_g = np.tile(np.arange(MISC_V[4]), MISC_V[1] * MISC_V[2])
    t_mA = np.concatenate([e_pri[pri_g], e_sub[sub_g], e_pat[pat_g],
                           np.zeros((NMA, 128 - 3 * DM), np.float32)], axis=1)
    sp_g = np.repeat(np.arange(MISC_V[0]), MISC_V[3])
    key_g = np.tile(np.arange(MISC_V[3]), MISC_V[0])
    t_mB = np.concatenate([e_sp[sp_g], e_key[key_g],
                           np.zeros((NMB, 128 - 2 * DM), np.float32)], axis=1)
    t_misc = np.concatenate([t_mA, t_mB], axis=0)                # [7410,128]

    t_pair = t_pair.astype(ml_dtypes.bfloat16)
    t_mye4 = t_mye4.astype(ml_dtypes.bfloat16)
    t_misc = t_misc.astype(ml_dtypes.bfloat16)

    # --- weights ---
    W1f = np.asarray(W1, np.float32)          # [272, 256]
    sl = {
        "my": W1f[0:64], "ally": W1f[64:128], "enem": W1f[128:192],
        "sp": W1f[192:208], "pri": W1f[208:224], "sub": W1f[224:240],
        "key": W1f[240:256], "pat": W1f[256:272],
    }
    z80 = np.zeros((80, 256), np.float32)
    z96 = np.zeros((96, 256), np.float32)
    stat = [
        np.concatenate([sl["ally"], sl["ally"]], axis=0),
        np.concatenate([sl["enem"], sl["enem"]], axis=0),
        np.concatenate([sl["my"], sl["enem"]], axis=0),
        np.concatenate([sl["pri"], sl["sub"], sl["pat"], z80], axis=0),
        np.concatenate([sl["sp"], sl["key"], z96], axis=0),
    ]
    w1_arr = np.zeros((5, 2, 128, 128), dtype=ml_dtypes.bfloat16)
    for k in range(5):
        for m in range(2):
            w1_arr[k, m] = stat[k][:, m * 128:(m + 1) * 128]
    w2_arr = np.ascontiguousarray(
        np.asarray(W2, np.float32).astype(ml_dtypes.bfloat16).reshape(2, 128, 128))
    w3_arr = np.asarray(W3, np.float32).astype(ml_dtypes.bfloat16)
    b1_arr = np.asarray(b1, np.float32).reshape(2, 128, 1)
    b2_arr = np.asarray(b2, np.float32).reshape(128, 1)
    b3_arr = np.asarray(b3, np.float32).reshape(1, 1)

    # --- indices ---
    al = _fix(np.asarray(ally), NCHAMP)
    en = _fix(np.asarray(enem), NCHAMP)
    myx = _fix(np.asarray(my_idx), NCHAMP)
    mi = np.asarray(misc_idx)
    mif = [_fix(mi[:, j], MISC_V[j]) for j in range(5)]

    def pairk(a, b):
        s = np.maximum(a, b)
        t = np.minimum(a, b)
        return s * (s + 1) // 2 + t

    kA01 = pairk(al[:, 0], al[:, 1])
    kA23 = pairk(al[:, 2], al[:, 3])
    kE01 = pairk(en[:, 0], en[:, 1])
    kE23 = pairk(en[:, 2], en[:, 3])
    kB = myx * NCHAMP + en[:, 4]
    kC1 = (mif[1] * MISC_V[2] + mif[2]) * MISC_V[4] + mif[4]
    kC2 = NMA + mif[0] * MISC_V[3] + mif[3]

    in_maps = []
    for c in range(N_CORES):
        s0 = c * B_CORE
        iA = np.empty((128, NSUP * (4 * G // 16)), np.int16)
        iB = np.empty((128, NSUP * (G // 16)), np.int16)
        iC = np.empty((128, NSUP * (2 * G // 16)), np.int16)
        for s in range(NSUP):
            r = slice(s0 + s * G, s0 + (s + 1) * G)
            iA[:, s * (4 * G // 16):(s + 1) * (4 * G // 16)] = _wrap16(
                np.concatenate([kA01[r], kA23[r], kE01[r], kE23[r]]))
            iB[:, s * (G // 16):(s + 1) * (G // 16)] = _wrap16(kB[r])
            iC[:, s * (2 * G // 16):(s + 1) * (2 * G // 16)] = _wrap16(
                np.concatenate([kC1[r], kC2[r]]))
        in_maps.append({
            "t_pair": t_pair, "t_mye4": t_mye4, "t_misc": t_misc,
            "idxA": iA, "idxB": iB, "idxC": iC,
            "w1": w1_arr, "w2": w2_arr, "w3": w3_arr,
            "b1": b1_arr, "b2": b2_arr, "b3": b3_arr,
        })
    return in_maps


def kernel(**inputs):
    if "nc" not in _COMPILED:
        _COMPILED["nc"] = _build_program()
    nc = _COMPILED["nc"]
    in_maps = _prep_inputs(**inputs)
    res = run_bass_kernel_spmd(nc, in_maps, core_ids=list(range(N_CORES)))
    out = np.concatenate([r["out"].reshape(B_CORE) for r in res.results])
    return out.astype(np.float32)


# revision 19
# speedup vs baseline: 3.5649x; 1.5623x over previous
"""Trainium2 Bass kernel for nn_CompMLP (embedding gathers + 3-layer MLP).

Strategy v4 (pure data parallel, 8 cores, B rows split evenly):
  - Champion lookups (the big tables) are DMA gathers (SWDGE) from HBM
    tables of 256-byte rows in transposed mode, landing feature-on-
    partition ready for matmul.  Five 512-index calls per 512-row tile,
    round-robined over all 4 SWDGE queues so all four Q7 core pairs
    generate descriptors concurrently (measured 1.7us per call):
      t_pair [14706, 128]: [emb_i | emb_j], sorted champ pairs
        (k = i(i+1)/2 + j, i >= j).  Serves a01/a23/e01/e23; pair-sum
        happens for free in PSUM accumulation via stacked W1 slices.
      t_mye4 [29241, 128]: [emb_my | emb_e4], k = my*171 + e4.
  - The five tiny misc tables go through an on-chip one-hot: one K=5
    matmul replicates the 5 misc index rows into per-partition slots
    (partition p of the chunk owns one (table, vocab) entry), DVE
    is_equal against a per-partition iota column produces the packed
    one-hot, and the h1 contribution comes from matmuls with
    host-premultiplied (emb_misc @ W1_slice) tables.  No GPSIMD cost.
  - MLP: 7 K-chunk matmuls -> 256-dim h1 (ScalarE fused bias+ReLU),
    2 matmuls -> h2, 1 matmul -> out scalar.
"""

import numpy as np
import ml_dtypes

import concourse.bass as bass  # noqa: F401
import concourse.mybir as mybir
from concourse import bacc
from concourse.tile import TileContext
from concourse.bass_utils import run_bass_kernel_spmd

# ---- problem constants (hardcoded per contract) ----
B_TOTAL = 262144
NCHAMP = 171
DC = 64
DM = 16
MISC_V = (33, 9, 9, 65, 65)
N_CORES = 8
B_CORE = B_TOTAL // N_CORES   # 32768

F = 512                       # rows per tile
T_TILES = B_CORE // F         # 64

NPAIR = NCHAMP * (NCHAMP + 1) // 2   # 14706 sorted pairs
NMYE4 = NCHAMP * NCHAMP              # 29241

# misc one-hot chunk 0 layout: sp | pri | sub | key | unused
M0_OFF = (0, 33, 42, 51)      # offsets of sp, pri, sub, key
M0_USED = 116
# chunk 1: pat
M1_USED = 65

BF16 = mybir.dt.bfloat16
F32 = mybir.dt.float32
I16 = mybir.dt.int16
AF = mybir.ActivationFunctionType
ALU = mybir.AluOpType

_COMPILED = {}


def _fix(x, n):
    return np.where(x < 0, n - 1, x).astype(np.int64)


def _wrap16(idx):
    """[N] index list -> [128, N//16] int16 wrapped in 16 partitions,
    replicated across the 8 GPSIMD cores (dma_gather index layout)."""
    n = idx.shape[0]
    w = idx.reshape(n // 16, 16).T.astype(np.int16)
    return np.tile(w, (8, 1))


def _build_program():
    nc = bacc.Bacc("TRN2", target_bir_lowering=False, debug=False,
                   num_devices=N_CORES, num_swdge_queues=4)

    tp_d = nc.dram_tensor("t_pair", [NPAIR, 128], BF16, kind="ExternalInput")
    tm_d = nc.dram_tensor("t_mye4", [NMYE4, 128], BF16, kind="ExternalInput")
    IC = T_TILES * (F // 16)   # idx cols per list (64*32)
    idx_d = [nc.dram_tensor(f"idx{j}", [128, IC], I16, kind="ExternalInput")
             for j in range(5)]
    mrow_d = nc.dram_tensor("mrow", [5, B_CORE], BF16, kind="ExternalInput")
    selw_d = nc.dram_tensor("selw", [2, 5, 128], BF16, kind="ExternalInput")
    iota_d = nc.dram_tensor("iota", [2, 128, 1], F32, kind="ExternalInput")
    w1_d = nc.dram_tensor("w1", [3, 2, 128, 128], BF16, kind="ExternalInput")
    wm_d = nc.dram_tensor("wm", [2, 2, 128, 128], BF16, kind="ExternalInput")
    w2_d = nc.dram_tensor("w2", [2, 128, 128], BF16, kind="ExternalInput")
    w3_d = nc.dram_tensor("w3", [128, 1], BF16, kind="ExternalInput")
    b1_d = nc.dram_tensor("b1", [2, 128, 1], F32, kind="ExternalInput")
    b2_d = nc.dram_tensor("b2", [128, 1], F32, kind="ExternalInput")
    b3_d = nc.dram_tensor("b3", [1, 1], F32, kind="ExternalInput")
    out_d = nc.dram_tensor("out", [T_TILES, F], F32, kind="ExternalOutput")

    with TileContext(nc) as tc:
        with (
            tc.tile_pool(name="const", bufs=1) as cpool,
            tc.tile_pool(name="gath", bufs=2) as gpool,
            tc.tile_pool(name="eqp", bufs=4) as epool,
            tc.tile_pool(name="act", bufs=3) as hpool,
            tc.tile_pool(name="outp", bufs=8) as opool,
            tc.tile_pool(name="ps1", bufs=3, space="PSUM") as ps1pool,
            tc.tile_pool(name="ps2", bufs=2, space="PSUM") as ps2pool,
            tc.tile_pool(name="psr", bufs=1, space="PSUM") as psrpool,
        ):
            idx_t = []
            for j in range(5):
                it = cpool.tile([128, IC], I16, tag=f"idx{j}", name=f"idx{j}")
                nc.sync.dma_start(out=it[:, :], in_=idx_d[j][:, :])
                idx_t.append(it)
            mrow_t = cpool.tile([5, B_CORE], BF16, tag="mrow")
            nc.sync.dma_start(out=mrow_t[:, :], in_=mrow_d[:, :])
            selw_t = [cpool.tile([5, 128], BF16, tag=f"selw{c}", name=f"selw{c}")
                      for c in range(2)]
            for c in range(2):
                nc.sync.dma_start(out=selw_t[c][:, :], in_=selw_d[c])
            iota_t = [cpool.tile([128, 1], F32, tag=f"iota{c}", name=f"iota{c}")
                      for c in range(2)]
            for c in range(2):
                nc.sync.dma_start(out=iota_t[c][:, :], in_=iota_d[c])
            w1_t = [[cpool.tile([128, 128], BF16, tag=f"w1_{k}_{m}",
                                name=f"w1_{k}_{m}") for m in range(2)]
                    for k in range(3)]
            for k in range(3):
                for m in range(2):
                    nc.sync.dma_start(out=w1_t[k][m][:, :], in_=w1_d[k, m])
            wm_t = [[cpool.tile([128, 128], BF16, tag=f"wm_{c}_{m}",
                                name=f"wm_{c}_{m}") for m in range(2)]
                    for c in range(2)]
            for c in range(2):
                for m in range(2):
                    nc.sync.dma_start(out=wm_t[c][m][:, :], in_=wm_d[c, m])
            w2_t = [cpool.tile([128, 128], BF16, tag=f"w2_{m}", name=f"w2_{m}")
                    for m in range(2)]
            for m in range(2):
                nc.sync.dma_start(out=w2_t[m][:, :], in_=w2_d[m])
            w3_t = cpool.tile([128, 1], BF16, tag="w3")
            nc.sync.dma_start(out=w3_t[:, :], in_=w3_d[:, :])
            b1_t = [cpool.tile([128, 1], F32, tag=f"b1_{m}", name=f"b1_{m}")
                    for m in range(2)]
            for m in range(2):
                nc.sync.dma_start(out=b1_t[m][:, :], in_=b1_d[m])
            b2_t = cpool.tile([128, 1], F32, tag="b2")
            nc.sync.dma_start(out=b2_t[:, :], in_=b2_d[:, :])
            b3_t = cpool.tile([1, 1], F32, tag="b3")
            nc.sync.dma_start(out=b3_t[:, :], in_=b3_d[:, :])

            # dma_gather reads its index tile on the Q7 at descriptor-gen
            # time; fence the idx uploads before the first gather.  The DVE
            # touches are engine instructions gated on the upload DMAs; the
            # Pool-side touch is gated on the DVE result, so the Pool queue
            # (and every dma_gather behind it) starts after the idx data is
            # resident.
            touch = cpool.tile([128, 1], I16, tag="touch")
            for j in range(5):
                nc.vector.tensor_scalar_add(touch[:, :], idx_t[j][:, 0:1], 0)
            touch2 = cpool.tile([128, 1], I16, tag="touch2")
            nc.gpsimd.tensor_scalar_add(touch2[:, :], touch[:, :], 0)

            IW = F // 16   # idx cols per tile (32)
            for t in range(T_TILES):
                g = []
                for j in range(5):
                    gt = gpool.tile([128, 1, F], BF16, tag=f"g{j}")
                    nc.gpsimd.dma_gather(
                        gt[:, :, :],
                        (tp_d if j < 4 else tm_d)[:, :],
                        idx_t[j][:, t * IW:(t + 1) * IW],
                        num_idxs=F, num_idxs_reg=F,
                        elem_size=128, transpose=True)
                    g.append(gt)

                eq = []
                for c in range(2):
                    rep = psrpool.tile([128, F], F32, tag=f"rep{c}")
                    nc.tensor.matmul(rep[:, :], selw_t[c][:, :],
                                     mrow_t[:, t * F:(t + 1) * F],
                                     start=True, stop=True)
                    eqc = epool.tile([128, F], BF16, tag=f"eq{c}")
                    nc.vector.tensor_scalar(eqc[:, :], rep[:, :],
                                            iota_t[c][:, 0:1], None,
                                            op0=ALU.is_equal)
                    eq.append(eqc)

                h1 = []
                for m in range(2):
                    ps = ps1pool.tile([128, F], F32, tag="ps1")
                    nc.tensor.matmul(ps[:, :], w1_t[0][m][:, :],
                                     g[0][:, 0, :], start=True, stop=False)
                    nc.tensor.matmul(ps[:, :], w1_t[0][m][:, :],
                                     g[1][:, 0, :], start=False, stop=False)
                    nc.tensor.matmul(ps[:, :], w1_t[1][m][:, :],
                                     g[2][:, 0, :], start=False, stop=False)
                    nc.tensor.matmul(ps[:, :], w1_t[1][m][:, :],
                                     g[3][:, 0, :], start=False, stop=False)
                    nc.tensor.matmul(ps[:, :], w1_t[2][m][:, :],
                                     g[4][:, 0, :], start=False, stop=False)
                    nc.tensor.matmul(ps[:, :], wm_t[0][m][:, :],
                                     eq[0][:, :], start=False, stop=False)
                    nc.tensor.matmul(ps[:, :], wm_t[1][m][:, :],
                                     eq[1][:, :], start=False, stop=True)
                    hm = hpool.tile([128, F], BF16, tag=f"h1_{m}")
                    nc.scalar.activation(hm[:, :], ps[:, :], AF.Relu,
                                         bias=b1_t[m][:, 0:1])
                    h1.append(hm)

                ps2 = ps2pool.tile([128, F], F32, tag="ps2")
                nc.tensor.matmul(ps2[:, :], w2_t[0][:, :], h1[0][:, :],
                                 start=True, stop=False)
                nc.tensor.matmul(ps2[:, :], w2_t[1][:, :], h1[1][:, :],
                                 start=False, stop=True)
                h2 = hpool.tile([128, F], BF16, tag="h2")
                nc.scalar.activation(h2[:, :], ps2[:, :], AF.Relu,
                                     bias=b2_t[:, 0:1])

                ps3 = psrpool.tile([1, F], F32, tag="ps3")
                nc.tensor.matmul(ps3[:, :], w3_t[:, 0:1], h2[:, :],
                                 start=True, stop=True)
                ot = opool.tile([1, F], F32, tag="ot")
                nc.scalar.activation(ot[:, :], ps3[:, :], AF.Identity,
                                     bias=b3_t[0:1, 0:1])
                nc.sync.dma_start(out=out_d[t:t + 1, :], in_=ot[:, :])

    # Post-lowering pass over the scheduled instruction stream:
    #   1. Spread the SWDGE gathers over all 4 queues (4 Q7 core pairs
    #      generate descriptors concurrently; measured ~2.9x).  A DMASW
    #      completion semaphore may only be incremented from one queue, so
    #      the queue is chosen per lane-sem (each distinct sem id maps to
    #      one queue, round-robin by first appearance).
    #   2. Throttle to at most ONE gather DMA in flight per queue by adding
    #      to each gather a wait on the completion sem value of the
    #      previous gather on the same queue (deeper pipelining corrupts
    #      descriptors - measured on HW).
    sem_queue: dict = {}
    sem_cum: dict = {}
    last_on_queue: dict = {}
    nextq = 0
    for blk in nc.m.functions[0].blocks:
        for inst in blk.instructions:
            if not isinstance(inst, mybir.InstDMAGatherAnt):
                continue
            si = inst.sync_info
            upd = [u for u in si.on_update
                   if u.sync_type == "semaphore" and u.update_mode == "sem-add-imm"]
            assert len(upd) == 1, upd
            sid = upd[0].id
            if sid not in sem_queue:
                sem_queue[sid] = nextq
                nextq = (nextq + 1) % 4
            q = sem_queue[sid]
            inst.queue_num = q
            if q in last_on_queue:
                prev_sid, prev_cum, prev_name = last_on_queue[q]
                si.on_wait = list(si.on_wait) + [mybir.SyncWait(
                    sync_type="semaphore", id=prev_sid,
                    wait_mode="sem-ge-imm", wait_value=prev_cum,
                    ant_name=prev_name)]
            sem_cum[sid] = sem_cum.get(sid, 0) + int(upd[0].update_value)
            last_on_queue[q] = (sid, sem_cum[sid], upd[0].ant_name)

    nc.compile()
    return nc


def _prep_inputs(my_idx, ally, enem, misc_idx, emb_champ, emb_sp, emb_pri,
                 emb_sub, emb_key, emb_pat, W1, b1, W2, b2, W3, b3):
    emb = np.asarray(emb_champ, np.float32)
    e_misc = [np.asarray(e, np.float32)
              for e in (emb_sp, emb_pri, emb_sub, emb_key, emb_pat)]

    # --- gather tables ---
    ii, jj = np.tril_indices(NCHAMP)
    t_pair = np.concatenate([emb[ii], emb[jj]], axis=1).astype(
        ml_dtypes.bfloat16)
    my_g = np.repeat(np.arange(NCHAMP), NCHAMP)
    e4_g = np.tile(np.arange(NCHAMP), NCHAMP)
    t_mye4 = np.concatenate([emb[my_g], emb[e4_g]], axis=1).astype(
        ml_dtypes.bfloat16)

    # --- weights ---
    W1f = np.asarray(W1, np.float32)          # [272, 256]
    sl_my, sl_al, sl_en = W1f[0:64], W1f[64:128], W1f[128:192]
    stat = [
        np.concatenate([sl_al, sl_al], axis=0),
        np.concatenate([sl_en, sl_en], axis=0),
        np.concatenate([sl_my, sl_en], axis=0),
    ]
    w1_arr = np.zeros((3, 2, 128, 128), dtype=ml_dtypes.bfloat16)
    for k in range(3):
        for m in range(2):
            w1_arr[k, m] = stat[k][:, m * 128:(m + 1) * 128]

    # misc premultiplied one-hot weights
    Wm_sl = [W1f[192:208], W1f[208:224], W1f[224:240], W1f[240:256],
             W1f[256:272]]
    M = [e_misc[s] @ Wm_sl[s] for s in range(5)]   # [(33|9|9|65|65), 256]
    wm0 = np.zeros((128, 256), np.float32)
    for s in range(4):
        wm0[M0_OFF[s]:M0_OFF[s] + M[s].shape[0]] = M[s]
    wm1 = np.zeros((128, 256), np.float32)
    wm1[:M1_USED] = M[4]
    wm_arr = np.zeros((2, 2, 128, 128), dtype=ml_dtypes.bfloat16)
    for m in range(2):
        wm_arr[0, m] = wm0[:, m * 128:(m + 1) * 128]
        wm_arr[1, m] = wm1[:, m * 128:(m + 1) * 128]

    selw = np.zeros((2, 5, 128), dtype=ml_dtypes.bfloat16)
    for s in range(4):
        selw[0, s, M0_OFF[s]:M0_OFF[s] + M[s].shape[0]] = 1
    selw[1, 4, :M1_USED] = 1
    iota = np.full((2, 128, 1), -1.0, np.float32)
    for s in range(4):
        n = M[s].shape[0]
        iota[0, M0_OFF[s]:M0_OFF[s] + n, 0] = np.arange(n)
    iota[1, :M1_USED, 0] = np.arange(M1_USED)

    w2_arr = np.ascontiguousarray(
        np.asarray(W2, np.float32).astype(ml_dtypes.bfloat16).reshape(
            2, 128, 128))
    w3_arr = np.asarray(W3, np.float32).astype(ml_dtypes.bfloat16)
    b1_arr = np.asarray(b1, np.float32).reshape(2, 128, 1)
    b2_arr = np.asarray(b2, np.float32).reshape(128, 1)
    b3_arr = np.asarray(b3, np.float32).reshape(1, 1)

    # --- indices ---
    al = _fix(np.asarray(ally), NCHAMP)
    en = _fix(np.asarray(enem), NCHAMP)
    myx = _fix(np.asarray(my_idx), NCHAMP)
    mi = np.asarray(misc_idx)
    mif = [_fix(mi[:, j], MISC_V[j]) for j in range(5)]

    def pairk(a, b):
        s = np.maximum(a, b)
        t = np.minimum(a, b)
        return s * (s + 1) // 2 + t

    lists = [pairk(al[:, 0], al[:, 1]), pairk(al[:, 2], al[:, 3]),
             pairk(en[:, 0], en[:, 1]), pairk(en[:, 2], en[:, 3]),
             myx * NCHAMP + en[:, 4]]
    mrow = np.stack([mif[s] for s in range(5)]).astype(ml_dtypes.bfloat16)

    in_maps = []
    for c in range(N_CORES):
        r = slice(c * B_CORE, (c + 1) * B_CORE)
        im = {
            "t_pair": t_pair, "t_mye4": t_mye4,
            "mrow": np.ascontiguousarray(mrow[:, r]),
            "selw": selw, "iota": iota,
            "w1": w1_arr, "wm": wm_arr, "w2": w2_arr, "w3": w3_arr,
            "b1": b1_arr, "b2": b2_arr, "b3": b3_arr,
        }
        for j in range(5):
            im[f"idx{j}"] = _wrap16(lists[j][r])
        in_maps.append(im)
    return in_maps


def kernel(**inputs):
    if "nc" not in _COMPILED:
        _COMPILED["nc"] = _build_program()
    nc = _COMPILED["nc"]
    in_maps = _prep_inputs(**inputs)
    res = run_bass_kernel_spmd(nc, in_maps, core_ids=list(range(N_CORES)))
    out = np.concatenate([r["out"].reshape(B_CORE) for r in res.results])
    return out.astype(np.float32)


# revision 20
# speedup vs baseline: 4.2711x; 1.1981x over previous
"""Trainium2 Bass kernel for nn_CompMLP (embedding gathers + 3-layer MLP).

Strategy v6 (pure data parallel, 8 cores, B rows split evenly):
  - The four champion-pair lookups are DMA gathers (SWDGE) from an HBM
    table of 256-byte rows in transposed mode, landing feature-on-
    partition ready for matmul: t_pair [14706, 128] holds [emb_i|emb_j]
    for sorted champ pairs (k = i(i+1)/2 + j, i >= j); pair-sum happens
    for free in PSUM accumulation via stacked W1 slices.  Four 512-index
    calls per 512-row tile, spread over all 4 SWDGE queues (all four Q7
    core pairs generate descriptors concurrently), throttled to one
    in-flight DMA per queue (deeper pipelining corrupts descriptors).
  - Everything with a small vocab (my, e4, and the five misc tables;
    523 one-hot rows over 5 chunks) goes through an on-chip one-hot:
    one K=7 matmul per chunk replicates the needed index rows into
    per-partition slots, DVE is_equal against a per-partition iota
    column produces the packed one-hot, and the h1 contribution comes
    from matmuls with host-premultiplied (emb @ W1_slice) tables.
  - MLP: 9 K-chunk matmuls -> 256-dim h1 (ScalarE fused bias+ReLU),
    2 matmuls -> h2, 1 matmul -> out scalar.
"""

import numpy as np
import ml_dtypes

import concourse.bass as bass  # noqa: F401
import concourse.mybir as mybir
from concourse import bacc
from concourse.tile import TileContext
from concourse.bass_utils import run_bass_kernel_spmd

# ---- problem constants (hardcoded per contract) ----
B_TOTAL = 262144
NCHAMP = 171
DC = 64
DM = 16
MISC_V = (33, 9, 9, 65, 65)
N_CORES = 8
B_CORE = B_TOTAL // N_CORES   # 32768

F = 512                       # rows per tile
T_TILES = B_CORE // F         # 64

NPAIR = NCHAMP * (NCHAMP + 1) // 2   # 14706 sorted pairs

# one-hot slots: (name, vocab) in packing order
OH_SIZES = (171, 171, 33, 9, 9, 65, 65)   # my, e4, sp, pri, sub, key, pat
OH_NSLOT = 7
OH_NCHUNK = 5


def _oh_segs():
    """Pack the 7 slot vocabularies into 128-partition chunks.
    Returns (slot, lo, hi, chunk, part_off) tuples."""
    segs = []
    chunk, off = 0, 0
    for s, size in enumerate(OH_SIZES):
        lo = 0
        while lo < size:
            take = min(128 - off, size - lo)
            segs.append((s, lo, lo + take, chunk, off))
            off += take
            lo += take
            if off == 128:
                chunk += 1
                off = 0
    return segs


BF16 = mybir.dt.bfloat16
F32 = mybir.dt.float32
I16 = mybir.dt.int16
AF = mybir.ActivationFunctionType
ALU = mybir.AluOpType

_COMPILED = {}


def _fix(x, n):
    return np.where(x < 0, n - 1, x).astype(np.int64)


def _wrap16(idx):
    """[N] index list -> [128, N//16] int16 wrapped in 16 partitions,
    replicated across the 8 GPSIMD cores (dma_gather index layout)."""
    n = idx.shape[0]
    w = idx.reshape(n // 16, 16).T.astype(np.int16)
    return np.tile(w, (8, 1))


def _build_program():
    nc = bacc.Bacc("TRN2", target_bir_lowering=False, debug=False,
                   num_devices=N_CORES, num_swdge_queues=4)

    tp_d = nc.dram_tensor("t_pair", [NPAIR, 128], BF16, kind="ExternalInput")
    IC = T_TILES * (F // 16)   # idx cols per list (64*32)
    idx_d = [nc.dram_tensor(f"idx{j}", [128, IC], I16, kind="ExternalInput")
             for j in range(4)]
    mrow_d = nc.dram_tensor("mrow", [OH_NSLOT, B_CORE], BF16,
                            kind="ExternalInput")
    selw_d = nc.dram_tensor("selw", [OH_NCHUNK, OH_NSLOT, 128], BF16,
                            kind="ExternalInput")
    iota_d = nc.dram_tensor("iota", [OH_NCHUNK, 128, 1], F32,
                            kind="ExternalInput")
    w1_d = nc.dram_tensor("w1", [2, 2, 128, 128], BF16, kind="ExternalInput")
    wm_d = nc.dram_tensor("wm", [OH_NCHUNK, 2, 128, 128], BF16,
                          kind="ExternalInput")
    w2_d = nc.dram_tensor("w2", [2, 128, 128], BF16, kind="ExternalInput")
    w3_d = nc.dram_tensor("w3", [128, 1], BF16, kind="ExternalInput")
    b1_d = nc.dram_tensor("b1", [2, 128, 1], F32, kind="ExternalInput")
    b2_d = nc.dram_tensor("b2", [128, 1], F32, kind="ExternalInput")
    b3_d = nc.dram_tensor("b3", [1, 1], F32, kind="ExternalInput")
    out_d = nc.dram_tensor("out", [T_TILES, F], F32, kind="ExternalOutput")

    with TileContext(nc) as tc:
        with (
            tc.tile_pool(name="const", bufs=1) as cpool,
            tc.tile_pool(name="gath", bufs=2) as gpool,
            tc.tile_pool(name="eqp", bufs=2) as epool,
            tc.tile_pool(name="act", bufs=3) as hpool,
            tc.tile_pool(name="outp", bufs=8) as opool,
            tc.tile_pool(name="ps1", bufs=3, space="PSUM") as ps1pool,
            tc.tile_pool(name="ps2", bufs=2, space="PSUM") as ps2pool,
            tc.tile_pool(name="psr", bufs=2, space="PSUM") as psrpool,
            tc.tile_pool(name="ps3", bufs=1, space="PSUM") as ps3pool,
        ):
            idx_t = []
            for j in range(4):
                it = cpool.tile([128, IC], I16, tag=f"idx{j}", name=f"idx{j}")
                nc.sync.dma_start(out=it[:, :], in_=idx_d[j][:, :])
                idx_t.append(it)
            mrow_t = cpool.tile([OH_NSLOT, B_CORE], BF16, tag="mrow")
            nc.sync.dma_start(out=mrow_t[:, :], in_=mrow_d[:, :])
            selw_t = [cpool.tile([OH_NSLOT, 128], BF16, tag=f"selw{c}",
                                 name=f"selw{c}") for c in range(OH_NCHUNK)]
            for c in range(OH_NCHUNK):
                nc.sync.dma_start(out=selw_t[c][:, :], in_=selw_d[c])
            iota_t = [cpool.tile([128, 1], F32, tag=f"iota{c}",
                                 name=f"iota{c}") for c in range(OH_NCHUNK)]
            for c in range(OH_NCHUNK):
                nc.sync.dma_start(out=iota_t[c][:, :], in_=iota_d[c])
            w1_t = [[cpool.tile([128, 128], BF16, tag=f"w1_{k}_{m}",
                                name=f"w1_{k}_{m}") for m in range(2)]
                    for k in range(2)]
            for k in range(2):
                for m in range(2):
                    nc.sync.dma_start(out=w1_t[k][m][:, :], in_=w1_d[k, m])
            wm_t = [[cpool.tile([128, 128], BF16, tag=f"wm_{c}_{m}",
                                name=f"wm_{c}_{m}") for m in range(2)]
                    for c in range(OH_NCHUNK)]
            for c in range(OH_NCHUNK):
                for m in range(2):
                    nc.sync.dma_start(out=wm_t[c][m][:, :], in_=wm_d[c, m])
            w2_t = [cpool.tile([128, 128], BF16, tag=f"w2_{m}", name=f"w2_{m}")
                    for m in range(2)]
            for m in range(2):
                nc.sync.dma_start(out=w2_t[m][:, :], in_=w2_d[m])
            w3_t = cpool.tile([128, 1], BF16, tag="w3")
            nc.sync.dma_start(out=w3_t[:, :], in_=w3_d[:, :])
            b1_t = [cpool.tile([128, 1], F32, tag=f"b1_{m}", name=f"b1_{m}")
                    for m in range(2)]
            for m in range(2):
                nc.sync.dma_start(out=b1_t[m][:, :], in_=b1_d[m])
            b2_t = cpool.tile([128, 1], F32, tag="b2")
            nc.sync.dma_start(out=b2_t[:, :], in_=b2_d[:, :])
            b3_t = cpool.tile([1, 1], F32, tag="b3")
            nc.sync.dma_start(out=b3_t[:, :], in_=b3_d[:, :])

            # dma_gather reads its index tile on the Q7 at descriptor-gen
            # time; fence the idx uploads before the first gather.
            touch = cpool.tile([128, 1], I16, tag="touch")
            for j in range(4):
                nc.vector.tensor_scalar_add(touch[:, :], idx_t[j][:, 0:1], 0)
            touch2 = cpool.tile([128, 1], I16, tag="touch2")
            nc.gpsimd.tensor_scalar_add(touch2[:, :], touch[:, :], 0)

            IW = F // 16   # idx cols per tile (32)
            for t in range(T_TILES):
                g = []
                for j in range(4):
                    gt = gpool.tile([128, 1, F], BF16, tag=f"g{j}")
                    nc.gpsimd.dma_gather(
                        gt[:, :, :], tp_d[:, :],
                        idx_t[j][:, t * IW:(t + 1) * IW],
                        num_idxs=F, num_idxs_reg=F,
                        elem_size=128, transpose=True)
                    g.append(gt)

                eq = []
                for c in range(OH_NCHUNK):
                    rep = psrpool.tile([128, F], F32, tag="rep")
                    nc.tensor.matmul(rep[:, :], selw_t[c][:, :],
                                     mrow_t[:, t * F:(t + 1) * F],
                                     start=True, stop=True)
                    eqc = epool.tile([128, F], BF16, tag=f"eq{c}")
                    nc.vector.tensor_scalar(eqc[:, :], rep[:, :],
                                            iota_t[c][:, 0:1], None,
                                            op0=ALU.is_equal)
                    eq.append(eqc)

                h1 = []
                for m in range(2):
                    ps = ps1pool.tile([128, F], F32, tag="ps1")
                    nc.tensor.matmul(ps[:, :], w1_t[0][m][:, :],
                                     g[0][:, 0, :], start=True, stop=False)
                    nc.tensor.matmul(ps[:, :], w1_t[0][m][:, :],
                                     g[1][:, 0, :], start=False, stop=False)
                    nc.tensor.matmul(ps[:, :], w1_t[1][m][:, :],
                                     g[2][:, 0, :], start=False, stop=False)
                    nc.tensor.matmul(ps[:, :], w1_t[1][m][:, :],
                                     g[3][:, 0, :], start=False, stop=False)
                    for c in range(OH_NCHUNK):
                        nc.tensor.matmul(ps[:, :], wm_t[c][m][:, :],
                                         eq[c][:, :], start=False,
                                         stop=(c == OH_NCHUNK - 1))
                    hm = hpool.tile([128, F], BF16, tag=f"h1_{m}")
                    nc.scalar.activation(hm[:, :], ps[:, :], AF.Relu,
                                         bias=b1_t[m][:, 0:1])
                    h1.append(hm)

                ps2 = ps2pool.tile([128, F], F32, tag="ps2")
                nc.tensor.matmul(ps2[:, :], w2_t[0][:, :], h1[0][:, :],
                                 start=True, stop=False)
                nc.tensor.matmul(ps2[:, :], w2_t[1][:, :], h1[1][:, :],
                                 start=False, stop=True)
                h2 = hpool.tile([128, F], BF16, tag="h2")
                nc.scalar.activation(h2[:, :], ps2[:, :], AF.Relu,
                                     bias=b2_t[:, 0:1])

                ps3 = ps3pool.tile([1, F], F32, tag="ps3")
                nc.tensor.matmul(ps3[:, :], w3_t[:, 0:1], h2[:, :],
                                 start=True, stop=True)
                ot = opool.tile([1, F], F32, tag="ot")
                nc.scalar.activation(ot[:, :], ps3[:, :], AF.Identity,
                                     bias=b3_t[0:1, 0:1])
                nc.sync.dma_start(out=out_d[t:t + 1, :], in_=ot[:, :])

    # Post-lowering pass over the scheduled instruction stream:
    #   1. Spread the SWDGE gathers over all 4 queues (4 Q7 core pairs
    #      generate descriptors concurrently).  A DMASW completion
    #      semaphore may only be incremented from one queue, so the queue
    #      is chosen per lane-sem (each distinct sem id maps to one
    #      queue, round-robin by first appearance).
    #   2. Throttle to at most ONE gather DMA in flight per queue by
    #      adding to each gather a wait on the completion sem value of
    #      the previous gather on the same queue (deeper pipelining
    #      corrupts descriptors - measured on HW).
    sem_queue: dict = {}
    sem_cum: dict = {}
    last_on_queue: dict = {}
    nextq = 0
    for blk in nc.m.functions[0].blocks:
        for inst in blk.instructions:
            if not isinstance(inst, mybir.InstDMAGatherAnt):
                continue
            si = inst.sync_info
            upd = [u for u in si.on_update
                   if u.sync_type == "semaphore"
                   and u.update_mode == "sem-add-imm"]
            assert len(upd) == 1, upd
            sid = upd[0].id
            if sid not in sem_queue:
                sem_queue[sid] = nextq
                nextq = (nextq + 1) % 4
            q = sem_queue[sid]
            inst.queue_num = q
            if q in last_on_queue:
                prev_sid, prev_cum, prev_name = last_on_queue[q]
                si.on_wait = list(si.on_wait) + [mybir.SyncWait(
                    sync_type="semaphore", id=prev_sid,
                    wait_mode="sem-ge-imm", wait_value=prev_cum,
                    ant_name=prev_name)]
            sem_cum[sid] = sem_cum.get(sid, 0) + int(upd[0].update_value)
            last_on_queue[q] = (sid, sem_cum[sid], upd[0].ant_name)

    nc.compile()
    return nc


def _prep_inputs(my_idx, ally, enem, misc_idx, emb_champ, emb_sp, emb_pri,
                 emb_sub, emb_key, emb_pat, W1, b1, W2, b2, W3, b3):
    emb = np.asarray(emb_champ, np.float32)
    e_misc = [np.asarray(e, np.float32)
              for e in (emb_sp, emb_pri, emb_sub, emb_key, emb_pat)]

    # --- gather table ---
    ii, jj = np.tril_indices(NCHAMP)
    t_pair = np.concatenate([emb[ii], emb[jj]], axis=1).astype(
        ml_dtypes.bfloat16)

    # --- weights ---
    W1f = np.asarray(W1, np.float32)          # [272, 256]
    sl_al, sl_en = W1f[64:128], W1f[128:192]
    stat = [np.concatenate([sl_al, sl_al], axis=0),
            np.concatenate([sl_en, sl_en], axis=0)]
    w1_arr = np.zeros((2, 2, 128, 128), dtype=ml_dtypes.bfloat16)
    for k in range(2):
        for m in range(2):
            w1_arr[k, m] = stat[k][:, m * 128:(m + 1) * 128]

    # one-hot premultiplied tables: my, e4, sp, pri, sub, key, pat
    M = [emb @ W1f[0:64],                    # my      [171, 256]
         emb @ W1f[128:192],                 # e4      [171, 256]
         e_misc[0] @ W1f[192:208],           # sp      [33, 256]
         e_misc[1] @ W1f[208:224],           # pri
         e_misc[2] @ W1f[224:240],           # sub
         e_misc[3] @ W1f[240:256],           # key
         e_misc[4] @ W1f[256:272]]           # pat
    segs = _oh_segs()
    wm = np.zeros((OH_NCHUNK, 128, 256), np.float32)
    selw = np.zeros((OH_NCHUNK, OH_NSLOT, 128), dtype=ml_dtypes.bfloat16)
    iota = np.full((OH_NCHUNK, 128, 1), -1.0, np.float32)
    for s, lo, hi, c, off in segs:
        n = hi - lo
        wm[c, off:off + n] = M[s][lo:hi]
        selw[c, s, off:off + n] = 1
        iota[c, off:off + n, 0] = np.arange(lo, hi)
    wm_arr = np.zeros((OH_NCHUNK, 2, 128, 128), dtype=ml_dtypes.bfloat16)
    for c in range(OH_NCHUNK):
        for m in range(2):
            wm_arr[c, m] = wm[c][:, m * 128:(m + 1) * 128]

    w2_arr = np.ascontiguousarray(
        np.asarray(W2, np.float32).astype(ml_dtypes.bfloat16).reshape(
            2, 128, 128))
    w3_arr = np.asarray(W3, np.float32).astype(ml_dtypes.bfloat16)
    b1_arr = np.asarray(b1, np.float32).reshape(2, 128, 1)
    b2_arr = np.asarray(b2, np.float32).reshape(128, 1)
    b3_arr = np.asarray(b3, np.float32).reshape(1, 1)

    # --- indices ---
    al = _fix(np.asarray(ally), NCHAMP)
    en = _fix(np.asarray(enem), NCHAMP)
    myx = _fix(np.asarray(my_idx), NCHAMP)
    mi = np.asarray(misc_idx)
    mif = [_fix(mi[:, j], MISC_V[j]) for j in range(5)]

    def pairk(a, b):
        s = np.maximum(a, b)
        t = np.minimum(a, b)
        return s * (s + 1) // 2 + t

    lists = [pairk(al[:, 0], al[:, 1]), pairk(al[:, 2], al[:, 3]),
             pairk(en[:, 0], en[:, 1]), pairk(en[:, 2], en[:, 3])]
    mrow = np.stack([myx, en[:, 4]] + mif).astype(ml_dtypes.bfloat16)

    in_maps = []
    for c in range(N_CORES):
        r = slice(c * B_CORE, (c + 1) * B_CORE)
        im = {
            "t_pair": t_pair,
            "mrow": np.ascontiguousarray(mrow[:, r]),
            "selw": selw, "iota": iota,
            "w1": w1_arr, "wm": wm_arr, "w2": w2_arr, "w3": w3_arr,
            "b1": b1_arr, "b2": b2_arr, "b3": b3_arr,
        }
        for j in range(4):
            im[f"idx{j}"] = _wrap16(lists[j][r])
        in_maps.append(im)
    return in_maps


def kernel(**inputs):
    if "nc" not in _COMPILED:
        _COMPILED["nc"] = _build_program()
    nc = _COMPILED["nc"]
    in_maps = _prep_inputs(**inputs)
    res = run_bass_kernel_spmd(nc, in_maps, core_ids=list(range(N_CORES)))
    out = np.concatenate([r["out"].reshape(B_CORE) for r in res.results])
    return out.astype(np.float32)
